# revision 2
# baseline (speedup 1.0000x reference)
"""Trainium2 Bass kernel v2 for nn_MlpwithSOMModule (retrieval_knn).

Differences vs v1 (452us baseline):
- fp16 everywhere: raw inputs cast to fp16 once (DVE), transposes run at
  1 cyc/row (vs fp32's 2), score matmuls at 1 cyc/row (vs fp32's 4), MLP
  in fp16 (same 1 cyc/row as f32r but single transpose evacuation).
  Numpy-validated: fp16 scores flip 18/32768 argmaxes -> rel_l2 1.1e-2
  (tolerance 2e-2); fp16 MLP alone is 4.4e-4.
- Tail restructured: L3 computed as column-matmuls (lhsT = 128-col slice
  of h2, rhs = w3 chunk, out [128,1]) so per-pair MLP outputs appear as
  columns directly; the ent-side gather is one matmul
  rent = onehotT.T @ o_entT.  Kills the row-broadcast (obc), the
  diagonal extraction, and the [1,N] PSUM tiles.
- PAIR=4 with two 512-col batches per weight: each (layer, j, c) weight
  chunk issues two back-to-back matmuls (batch A then B) so any
  walrus/PE stationary-weight reuse shortens the refill bubble.
- PSUM packed to exactly 8 banks via memset + start=False accumulation
  (zero-region is 2KB/bank, so independent small accumulators share a
  bank only if nothing ever issues start=True into it).
"""

from contextlib import ExitStack

import numpy as np

import concourse.bacc as bacc
import concourse.mybir as mybir
import concourse.tile as tile
from concourse.bass_utils import run_bass_kernel_spmd
from concourse.masks import make_identity

B, K, L, D = 4, 64, 128, 768
N_CORES = 8
BK = B * K                      # 256 (b,k) pairs total
BK_PER_CORE = BK // N_CORES     # 32
PAIR = 4                        # pairs per iteration
DC = D // 128                   # 6 contraction chunks
NQ = PAIR * 2                   # 8 operands (ctx/ent per pair)
NCOL = NQ * 128                 # 1024 columns per iteration
NB = 512                        # columns per matmul batch (PSUM bank limit)

F32 = mybir.dt.float32
F32R = mybir.dt.float32r
F16 = mybir.dt.float16


def build_kernel(n_bk: int = BK_PER_CORE):
    assert n_bk % PAIR == 0
    nc = bacc.Bacc("TRN2", target_bir_lowering=False)

    x = nc.declare_dram_parameter("x", [n_bk, 2, L, D], F32, isOutput=False)
    w1 = nc.declare_dram_parameter("w1", [D, D], F32, isOutput=False)
    b1 = nc.declare_dram_parameter("b1", [D], F32, isOutput=False)
    w2 = nc.declare_dram_parameter("w2", [D, D], F32, isOutput=False)
    b2 = nc.declare_dram_parameter("b2", [D], F32, isOutput=False)
    w3 = nc.declare_dram_parameter("w3", [D, 1], F32, isOutput=False)
    b3 = nc.declare_dram_parameter("b3", [1], F32, isOutput=False)
    out = nc.declare_dram_parameter("out", [n_bk, L], F32, isOutput=True)

    with tile.TileContext(nc) as tc:
        with ExitStack() as ctx:
            _emit(ctx, tc, n_bk, x, w1, b1, w2, b2, w3, b3, out)
    nc.compile()
    return nc


def _emit(ctx, tc, n_bk, x, w1, b1, w2, b2, w3, b3, out):
    nc = tc.nc
    AF = mybir.ActivationFunctionType
    ALU = mybir.AluOpType

    consts = ctx.enter_context(tc.tile_pool(name="consts", bufs=1))
    raw = ctx.enter_context(tc.tile_pool(name="raw", bufs=1))
    r16 = ctx.enter_context(tc.tile_pool(name="r16", bufs=2))
    xt = ctx.enter_context(tc.tile_pool(name="xt", bufs=2))
    hp = ctx.enter_context(tc.tile_pool(name="hp", bufs=2))
    small = ctx.enter_context(tc.tile_pool(name="small", bufs=4))
    # PSUM: exactly 8 banks
    pA = ctx.enter_context(tc.tile_pool(name="pA", bufs=2, space="PSUM"))    # 2
    pB = ctx.enter_context(tc.tile_pool(name="pB", bufs=2, space="PSUM"))    # 2
    ptr = ctx.enter_context(tc.tile_pool(name="ptr", bufs=2, space="PSUM"))  # 2
    psc = ctx.enter_context(tc.tile_pool(name="psc", bufs=1, space="PSUM"))  # 1
    pl3 = ctx.enter_context(tc.tile_pool(name="pl3", bufs=1, space="PSUM"))  # 1

    n_iter = n_bk // PAIR

    # ---- constants / weights (DMAs emitted after iter-0 raw loads) ----
    b1_sb = consts.tile([128, DC], F32)
    b2_sb = consts.tile([128, DC], F32)
    b3_sb = consts.tile([1, 1], F32)

    w1_st = consts.tile([128, DC, D], F32)
    w2_st = consts.tile([128, DC, D], F32)
    w3_st = consts.tile([128, DC], F32)
    w1_16 = consts.tile([128, DC, D], F16)
    w2_16 = consts.tile([128, DC, D], F16)
    w3_16 = consts.tile([128, DC], F16)

    def emit_weight_loads():
        nc.sync.dma_start(out=b1_sb, in_=b1.rearrange("(c p) -> p c", p=128))
        nc.sync.dma_start(out=b2_sb, in_=b2.rearrange("(c p) -> p c", p=128))
        nc.sync.dma_start(out=b3_sb, in_=b3[:].unsqueeze(0))
        nc.sync.dma_start(out=w1_st, in_=w1.rearrange("(c p) j -> p c j", p=128))
        nc.sync.dma_start(out=w2_st, in_=w2.rearrange("(c p) j -> p c j", p=128))
        nc.sync.dma_start(out=w3_st, in_=w3.rearrange("(c p) one -> p (c one)", p=128))
        nc.vector.tensor_copy(w1_16, w1_st)
        nc.vector.tensor_copy(w2_16, w2_st)
        nc.vector.tensor_copy(w3_16, w3_st)
        nc.vector.tensor_copy(b3_16, b3_sb)

    ident = consts.tile([128, 128], F32)
    make_identity(nc, ident)
    ident16 = consts.tile([128, 128], F16)
    nc.vector.tensor_copy(ident16, ident)

    # broadcast 2*b3 to all partitions: ones column x b3 via fp16 matmul
    # (b3 ~ 0.02; fp16 rounding is ~1e-5 absolute, negligible)
    ones_f = consts.tile([1, 128], F32)
    nc.vector.memset(ones_f, 1.0)
    ones16 = consts.tile([1, 128], F16)
    nc.vector.tensor_copy(ones16, ones_f)
    b3_16 = consts.tile([1, 1], F16)
    b3bc2 = consts.tile([128, 1], F32)

    def emit_b3_broadcast():
        # start=True is safe here: runs before any iteration uses this bank
        bb = pl3.tile([128, 16], F32, tag="l3", name="b3bc_ps")
        nc.tensor.matmul(bb[:, 0:1], lhsT=ones16, rhs=b3_16, start=True, stop=True)
        nc.vector.tensor_scalar(
            out=b3bc2, in0=bb[:, 0:1], scalar1=2.0, scalar2=None, op0=ALU.mult
        )

    res_all = consts.tile([128, n_bk], F32)

    # ---- per-iteration pieces ----
    def emit_load(it):
        tiles = []
        for q in range(NQ):
            rq = raw.tile([128, D], F32, tag="raw", bufs=2 * NQ, name=f"raw_{it}_{q}")
            nc.sync.dma_start(out=rq, in_=x[it * PAIR + q // 2, q % 2])
            tiles.append(rq)
        return tiles

    def emit_casts(it, raw_t):
        # per-q tiles on the (idle) GpSimd engine: transposes for operand q
        # start as soon as q's DMA+cast land, and the DVE evacuation queue
        # never stalls behind 768-wide casts
        x16 = []
        for q in range(NQ):
            xq = r16.tile([128, D], F16, tag="r16", bufs=2 * NQ, name=f"r16_{it}_{q}")
            nc.vector.tensor_copy(xq, raw_t[q])
            x16.append(xq)
        return x16

    def emit_transposes(it, x16):
        # xt16[d_part, c, q*128+l] = x16[q][l, c*128+d]
        # 4 transposes share one zeroed fp16 bank (start=False accumulate),
        # evacuated with a single [128,512] copy: 4x fewer DVE round-trips
        xt16 = xt.tile([128, DC, NCOL], F16, tag="xt", name=f"xt_{it}")
        for c in range(DC):
            for qg in range(2):
                tp = ptr.tile([128, 512], F16, tag="tr", name=f"tr_{it}_{c}_{qg}")
                nc.vector.memset(tp, 0.0)
                for qi in range(4):
                    q = qg * 4 + qi
                    nc.tensor.matmul(
                        tp[:, qi * 128 : (qi + 1) * 128],
                        lhsT=x16[q][:, c * 128 : (c + 1) * 128],
                        rhs=ident16,
                        is_transpose=True,
                        start=False,
                        stop=True,
                        skip_group_check=True,
                    )
                nc.vector.tensor_copy(xt16[:, c, qg * 512 : (qg + 1) * 512], tp)
        return xt16

    def emit_scores(it, xt16):
        # per pair: 6 accumulating fp16 matmuls into a shared zeroed bank
        sc = psc.tile([128, PAIR * 128], F32, tag="sc", name=f"sc_{it}")
        nc.vector.memset(sc, 0.0)
        onehots = []
        for p in range(PAIR):
            for c in range(DC):
                nc.tensor.matmul(
                    sc[:, p * 128 : (p + 1) * 128],
                    lhsT=xt16[:, c, (2 * p) * 128 : (2 * p + 1) * 128],
                    rhs=xt16[:, c, (2 * p + 1) * 128 : (2 * p + 2) * 128],
                    start=False,
                    stop=(c == DC - 1),
                    skip_group_check=True,
                )
            rm = small.tile([128, 1], F32, tag="rm", name=f"rm_{it}_{p}")
            nc.vector.reduce_max(rm, sc[:, p * 128 : (p + 1) * 128], axis=mybir.AxisListType.X)
            oh = small.tile([128, 128], F16, tag="oh", bufs=2 * PAIR, name=f"oh_{it}_{p}")
            nc.vector.tensor_scalar(
                out=oh,
                in0=sc[:, p * 128 : (p + 1) * 128],
                scalar1=rm,
                scalar2=None,
                op0=ALU.is_equal,
            )
            onehots.append(oh)
        return onehots

    def emit_mlp_layer(it, lname, src_t, w_16, b_sb):
        # dst[j, col] = relu(sum_c W[c,j].T @ src[c] + b); two 512-col
        # batches back-to-back per weight chunk
        dst_t = hp.tile([128, DC, NCOL], F16, tag=f"h{lname}", name=f"h{lname}_{it}")
        for j in range(DC):
            mmA = pA.tile([128, NB], F32, tag="mmA", name=f"mm{lname}A_{it}_{j}")
            mmB = pB.tile([128, NB], F32, tag="mmB", name=f"mm{lname}B_{it}_{j}")
            for c in range(DC):
                wa = w_16[:, c, j * 128 : (j + 1) * 128]
                nc.tensor.matmul(
                    mmA, lhsT=wa, rhs=src_t[:, c, 0:NB],
                    start=(c == 0), stop=(c == DC - 1),
                )
                nc.tensor.matmul(
                    mmB, lhsT=wa, rhs=src_t[:, c, NB:NCOL],
                    start=(c == 0), stop=(c == DC - 1),
                )
            nc.scalar.activation(
                out=dst_t[:, j, 0:NB], in_=mmA, func=AF.Relu, bias=b_sb[:, j : j + 1]
            )
            nc.scalar.activation(
                out=dst_t[:, j, NB:NCOL], in_=mmB, func=AF.Relu, bias=b_sb[:, j : j + 1]
            )
        return dst_t

    def emit_onehot_T(it, onehots):
        # all 4 onehot transposes share one zeroed bank, single evacuation
        tp = ptr.tile([128, 512], F16, tag="tr", name=f"ohT_{it}")
        nc.vector.memset(tp, 0.0)
        for p in range(PAIR):
            nc.tensor.matmul(
                tp[:, p * 128 : (p + 1) * 128],
                lhsT=onehots[p],
                rhs=ident16,
                is_transpose=True,
                start=False,
                stop=True,
                skip_group_check=True,
            )
        oht = small.tile([128, 512], F16, tag="ohT", bufs=2, name=f"ohTs_{it}")
        nc.vector.tensor_copy(oht, tp)
        return oht

    def emit_tail(it, h2_t, oht):
        # L3 as column-matmuls into a shared zeroed bank, then the gather
        # as rent = onehotT.T @ o_entT; res = rctx + rent + 2*b3
        l3 = pl3.tile([128, 16], F32, tag="l3", name=f"l3_{it}")
        nc.vector.memset(l3, 0.0)
        for g in range(NQ):
            for c in range(DC):
                nc.tensor.matmul(
                    l3[:, g : g + 1],
                    lhsT=h2_t[:, c, g * 128 : (g + 1) * 128],
                    rhs=w3_16[:, c : c + 1],
                    start=False,
                    stop=(c == DC - 1),
                    skip_group_check=True,
                )
        for p in range(PAIR):
            oe = small.tile([128, 1], F16, tag="oe", bufs=2 * PAIR, name=f"oe_{it}_{p}")
            nc.vector.tensor_copy(oe, l3[:, 2 * p + 1 : 2 * p + 2])
            nc.tensor.matmul(
                l3[:, 8 + p : 9 + p],
                lhsT=oht[:, p * 128 : (p + 1) * 128],
                rhs=oe,
                start=False,
                stop=True,
                skip_group_check=True,
            )
            col = it * PAIR + p
            # single PSUM input per DVE op (walrus PSUMInputs==1): rctx via
            # SBUF, then res = (rent + rctx) + 2*b3 in one tensor_scalar
            rc = small.tile([128, 1], F32, tag="rc", bufs=2 * PAIR, name=f"rc_{it}_{p}")
            nc.vector.tensor_copy(rc, l3[:, 2 * p : 2 * p + 1])
            nc.vector.tensor_scalar(
                out=res_all[:, col : col + 1],
                in0=l3[:, 8 + p : 9 + p],
                scalar1=rc,
                scalar2=b3bc2,
                op0=ALU.add,
                op1=ALU.add,
            )

    # ---- software pipeline ----
    # window i: [ohT(i-1)] [tr(i) 48] [scores(i) 24] [L2(i-1) 72] [L1(i) 72]
    #           [L3+rent(i-1) 52]
    state = {}
    prev = None
    raw_next = emit_load(0)
    emit_weight_loads()
    emit_b3_broadcast()
    for it in range(n_iter):
        raw_t = raw_next
        x16 = emit_casts(it, raw_t)
        if it + 1 < n_iter:
            raw_next = emit_load(it + 1)
        if prev is not None:
            state[prev]["ohts"] = emit_onehot_T(prev, state[prev]["oh"])
        xt16 = emit_transposes(it, x16)
        onehots = emit_scores(it, xt16)
        if prev is not None:
            h2 = emit_mlp_layer(prev, "l2", state[prev]["h1"], w2_16, b2_sb)
            state[prev]["h2"] = h2
        h1 = emit_mlp_layer(it, "l1", xt16, w1_16, b1_sb)
        if prev is not None:
            emit_tail(prev, state[prev]["h2"], state[prev]["ohts"])
            del state[prev]
        state[it] = {"h1": h1, "oh": onehots}
        prev = it
    # epilogue
    state[prev]["ohts"] = emit_onehot_T(prev, state[prev]["oh"])
    h2 = emit_mlp_layer(prev, "l2", state[prev]["h1"], w2_16, b2_sb)
    emit_tail(prev, h2, state[prev]["ohts"])

    # ---- store ----
    res_ps = pl3.tile([n_bk, 128], F32, tag="l3", name="res_ps")
    nc.tensor.transpose(res_ps, res_all, ident)
    res_T = small.tile([n_bk, 128], F32, tag="resT", name="res_T")
    nc.vector.tensor_copy(res_T, res_ps)
    nc.sync.dma_start(out=out[:, :], in_=res_T)


_NC_CACHE = {}


def _get_nc(n_bk):
    if n_bk not in _NC_CACHE:
        _NC_CACHE[n_bk] = build_kernel(n_bk)
    return _NC_CACHE[n_bk]


def run(inputs, trace=False):
    context = np.ascontiguousarray(np.asarray(inputs["context"], dtype=np.float32))
    xs = context.reshape(BK, 2, L, D)
    shared = {
        "w1": np.ascontiguousarray(np.asarray(inputs["W1"], dtype=np.float32)),
        "b1": np.ascontiguousarray(np.asarray(inputs["b1"], dtype=np.float32)),
        "w2": np.ascontiguousarray(np.asarray(inputs["W2"], dtype=np.float32)),
        "b2": np.ascontiguousarray(np.asarray(inputs["b2"], dtype=np.float32)),
        "w3": np.ascontiguousarray(np.asarray(inputs["W3"], dtype=np.float32)),
        "b3": np.ascontiguousarray(np.asarray(inputs["b3"], dtype=np.float32)),
    }
    in_maps = [
        {"x": np.ascontiguousarray(xs[c * BK_PER_CORE : (c + 1) * BK_PER_CORE]), **shared}
        for c in range(N_CORES)
    ]
    nc = _get_nc(BK_PER_CORE)
    res = run_bass_kernel_spmd(nc, in_maps, list(range(N_CORES)), trace=trace)
    outs = [m["out"] for m in res.results]
    full = np.concatenate(outs, axis=0).reshape(B, K, L).astype(np.float32)
    return full, res


def kernel(**inputs) -> np.ndarray:
    full, _ = run(inputs, trace=False)
    return full
